# revision 11
# baseline (speedup 1.0000x reference)
"""Trainium2 Bass kernel for dynamic kNN graph construction.

Reference computation (see problem):
  d2[i,j] = |src_i|^2 - 2 src_i.dst_j + |dst_j|^2   (fp32, [16384,16384] via GEMM)
  idx     = top-16 nearest dst per src row (ascending d2, stable ties)
  likelihood[e] = src_i . dst_j per edge, BatchNorm1d over all edges,
  exp -> edge weights, normalized by per-src-row degree sum.

Device strategy (8 NeuronCores, data-parallel over src rows, 2048 rows/core):
  - PE: bf16 hi/lo split GEMM (3 accumulating matmuls) + a K=6 matmul that
    folds -a/2 (per-row) and -c/2 (per-col, 3-term bf16 splits) into PSUM,
    so PSUM holds s~ = dot~ - a/2 - c/2 (~= -d2/2 up to ~1e-4)
  - ACT: plain PSUM -> SBUF copy
  - DVE: per-2048-chunk max8 + max_index -> 64 exact-f32 candidates
    (value + in-chunk idx) per row
  - host: merge 64 candidates -> top-16 with jax.lax.top_k tie semantics.
    Rows where device numerics could diverge from the reference are
    recomputed exactly: (a) near-ties (adjacent top-18 d2 gap < 5e-4,
    ~10x the worst-case device-vs-XLA numeric difference), (b) rows where
    one chunk supplied >=8 of the top-16 (candidate-truncation risk, makes
    the 8-per-chunk selection sound at any chunk width). For those rows the
    full d2 row is recomputed with an XLA-CPU matmul over gathered rows
    (M>=64 row-gathered matmuls are bitwise-identical to the reference's
    full [16384,16384] matmul - verified empirically).
  - host: replay the O(E) tail (gather-dot einsum, batchnorm, exp, degree
    normalization) with the same XLA-CPU ops the reference uses -> output
    is bitwise identical to the reference.
"""

import os
import sys

for _p in ("/opt/trn_rl_repo", "/root/.axon_site/_ro/trn_rl_repo"):
    if os.path.isdir(_p) and _p not in sys.path:
        sys.path.append(_p)

import numpy as np

N_SRC = 16384
N_DST = 16384
D = 128
K = 16
NCORES = 8
ROWS = N_SRC // NCORES          # 2048 src rows per core
NT = ROWS // 128                # 16 row-tiles of 128 rows
NJ = N_DST // 512               # 32 j-blocks of 512 cols
CHUNK = 2048                    # top-8 selection chunk width
NCH = N_DST // CHUNK            # 8 chunks per row
NCAND = NCH * 8                 # 64 candidates per row

BN_EPS = 1e-5

_cached_nc = None
last_exec_time_ns = None
last_results = None
last_flag_count = None


def _build_nc():
    """bf16 hi/lo split GEMM version.

    s~[i,j] = dot~[i,j] - a_i/2 - c_j/2 accumulated in one PSUM group:
      MM1 srcHi.T @ dstHi, MM2 srcHi.T @ dstLo, MM3 srcLo.T @ dstHi   (K=128)
      MM4 aug_lhs.T @ aug_rhs (K=6): rows 0-2 carry -c/2 (3-term bf16 split,
          ones weights), rows 3-5 carry -a/2 (3-term split, ones rhs)
    |s~ - s_exact| <= ~6e-5 (missing lo*lo term + split residuals), well under
    the host recheck margin RECHECK_DELTA=5e-4.
    ACT drains PSUM -> SBUF (plain copy); DVE does per-512-chunk max8 +
    max_index only.
    """
    global _cached_nc
    if _cached_nc is not None:
        return _cached_nc

    import concourse.bacc as bacc
    import concourse.tile as tile
    from concourse import mybir

    nc = bacc.Bacc("TRN2", target_bir_lowering=False, debug=False)
    bf16 = mybir.dt.bfloat16
    srcT_hi = nc.dram_tensor("srcT_hi", [D, ROWS], bf16, kind="ExternalInput").ap()
    srcT_lo = nc.dram_tensor("srcT_lo", [D, ROWS], bf16, kind="ExternalInput").ap()
    dstT_hi = nc.dram_tensor("dstT_hi", [D, N_DST], bf16, kind="ExternalInput").ap()
    dstT_lo = nc.dram_tensor("dstT_lo", [D, N_DST], bf16, kind="ExternalInput").ap()
    aug_lhs = nc.dram_tensor("aug_lhs", [6, ROWS], bf16, kind="ExternalInput").ap()
    aug_rhs = nc.dram_tensor("aug_rhs", [6, N_DST], bf16, kind="ExternalInput").ap()
    cval = nc.dram_tensor("cval", [NT, 128, NCAND], mybir.dt.float32, kind="ExternalOutput").ap()
    cidx = nc.dram_tensor("cidx", [NT, 128, NCAND], mybir.dt.uint32, kind="ExternalOutput").ap()

    with tile.TileContext(nc) as tc:
        with (
            tc.tile_pool(name="const", bufs=1) as cp,
            tc.tile_pool(name="psum", bufs=8, space="PSUM") as psp,
            tc.tile_pool(name="s", bufs=4) as sp,
            tc.tile_pool(name="cand", bufs=2) as cdp,
        ):
            sh_t = cp.tile([D, ROWS], bf16)
            sl_t = cp.tile([D, ROWS], bf16)
            dh_t = cp.tile([D, N_DST], bf16)
            dl_t = cp.tile([D, N_DST], bf16)
            al_t = cp.tile([128, ROWS], bf16)      # rows 0-5 used
            ar_t = cp.tile([128, N_DST], bf16)     # rows 0-5 used

            nc.sync.dma_start(sh_t[:], srcT_hi[:])
            nc.sync.dma_start(sl_t[:], srcT_lo[:])
            nc.sync.dma_start(al_t[0:6, :], aug_lhs[:])
            nc.sync.dma_start(ar_t[0:6, :], aug_rhs[:])
            NSPLIT = 8
            w = N_DST // NSPLIT
            for i in range(NSPLIT):
                sl = slice(i * w, (i + 1) * w)
                nc.sync.dma_start(dh_t[:, sl], dstT_hi[:, sl])
                nc.sync.dma_start(dl_t[:, sl], dstT_lo[:, sl])

            for t in range(NT):
                cv = cdp.tile([128, NCAND], mybir.dt.float32, name="cv", tag="cv")
                ci = cdp.tile([128, NCAND], mybir.dt.uint32, name="ci", tag="ci")
                tsl = slice(t * 128, (t + 1) * 128)
                # process j-blocks in groups of 8 (one PSUM bank each) so each
                # stationary operand is loaded once per group of 8 matmuls
                for g in range(NJ // 8):
                    pss = []
                    jbs = list(range(g * 8, g * 8 + 8))
                    for jb in jbs:
                        pss.append(psp.tile([128, 512], mybir.dt.float32, name="ps", tag="ps"))
                    for jb, ps in zip(jbs, pss):
                        jsl = slice(jb * 512, (jb + 1) * 512)
                        nc.tensor.matmul(ps[:], sh_t[:, tsl], dh_t[:, jsl],
                                         start=True, stop=False)
                    for jb, ps in zip(jbs, pss):
                        jsl = slice(jb * 512, (jb + 1) * 512)
                        nc.tensor.matmul(ps[:], sh_t[:, tsl], dl_t[:, jsl],
                                         start=False, stop=False, skip_group_check=True)
                    for jb, ps in zip(jbs, pss):
                        jsl = slice(jb * 512, (jb + 1) * 512)
                        nc.tensor.matmul(ps[:], sl_t[:, tsl], dh_t[:, jsl],
                                         start=False, stop=False, skip_group_check=True)
                    for jb, ps in zip(jbs, pss):
                        jsl = slice(jb * 512, (jb + 1) * 512)
                        nc.tensor.matmul(ps[:], al_t[0:6, tsl], ar_t[0:6, jsl],
                                         start=False, stop=True, skip_group_check=True)
                    # 8 j-blocks -> two 2048-wide selection chunks
                    sjs = []
                    for half in range(2):
                        sjs.append(sp.tile([128, CHUNK], mybir.dt.float32, name="sj", tag="sj"))
                    for bi, (jb, ps) in enumerate(zip(jbs, pss)):
                        sj = sjs[bi // 4]
                        nc.scalar.activation(sj[:, (bi % 4) * 512:(bi % 4) * 512 + 512],
                                             ps[:], mybir.ActivationFunctionType.Copy)
                    for half in range(2):
                        ch = g * 2 + half
                        csl = slice(ch * 8, ch * 8 + 8)
                        nc.vector.max(cv[:, csl], sjs[half][:])
                        nc.vector.max_index(ci[:, csl], cv[:, csl], sjs[half][:])
                nc.sync.dma_start(cval[t], cv[:])
                nc.sync.dma_start(cidx[t], ci[:])

    nc.compile()
    _cached_nc = nc
    return nc


def _split_bf16(x, terms):
    import ml_dtypes
    parts = []
    r = x.astype(np.float32)
    for _ in range(terms):
        h = r.astype(ml_dtypes.bfloat16)
        parts.append(h)
        r = r - h.astype(np.float32)
    return parts


def _cpu_device():
    import jax
    return jax.devices("cpu")[0]


def kernel(src_embeddings, dst_embeddings, gamma, beta, k):
    global last_exec_time_ns, last_results

    import jax
    import jax.numpy as jnp
    from concourse.bass_utils import run_bass_kernel_spmd

    src = np.asarray(src_embeddings, dtype=np.float32)
    dst = np.asarray(dst_embeddings, dtype=np.float32)
    gamma = np.asarray(gamma, dtype=np.float32)
    beta = np.asarray(beta, dtype=np.float32)
    kk = int(np.asarray(k))
    assert src.shape == (N_SRC, D) and dst.shape == (N_DST, D) and kk == K

    cpu = _cpu_device()
    with jax.default_device(cpu):
        a = np.asarray(jnp.sum(jnp.asarray(src) * jnp.asarray(src), axis=-1))
        c = np.asarray(jnp.sum(jnp.asarray(dst) * jnp.asarray(dst), axis=-1))

    dstT = np.ascontiguousarray(dst.T)
    d_hi, d_lo = _split_bf16(dstT, 2)
    nc1, nc2, nc3 = _split_bf16(-0.5 * c, 3)
    import ml_dtypes
    ones_n = np.ones((1, N_DST), ml_dtypes.bfloat16)
    aug_rhs = np.ascontiguousarray(
        np.concatenate([nc1[None, :], nc2[None, :], nc3[None, :],
                        ones_n, ones_n, ones_n], axis=0))

    in_maps = []
    for ci_ in range(NCORES):
        rows = slice(ci_ * ROWS, (ci_ + 1) * ROWS)
        srcT_c = np.ascontiguousarray(src[rows].T)
        s_hi, s_lo = _split_bf16(srcT_c, 2)
        na1, na2, na3 = _split_bf16(-0.5 * a[rows], 3)
        ones_r = np.ones((1, ROWS), ml_dtypes.bfloat16)
        aug_lhs = np.ascontiguousarray(
            np.concatenate([ones_r, ones_r, ones_r,
                            na1[None, :], na2[None, :], na3[None, :]], axis=0))
        in_maps.append({
            "srcT_hi": s_hi, "srcT_lo": s_lo,
            "dstT_hi": d_hi, "dstT_lo": d_lo,
            "aug_lhs": aug_lhs, "aug_rhs": aug_rhs,
        })

    nc = _build_nc()
    trace = bool(os.environ.get("KNN_BASS_TRACE"))
    res = run_bass_kernel_spmd(nc, in_maps, core_ids=list(range(NCORES)), trace=trace)
    last_exec_time_ns = res.exec_time_ns
    last_results = res

    # --- host: merge NCAND candidates -> top-16 per row ---
    # Device values are s~ ~= -d2/2 (device-vs-reference difference <~1e-4 in
    # d2 units). Flag rows whose top-18 has any adjacent d2 gap <
    # RECHECK_DELTA (plus rows where one chunk contributed >=8 of the top 16
    # - candidate truncation risk) and recompute those rows' d2
    # bitwise-as-reference on XLA-CPU (row-gathered matmuls with M>=64 are
    # bitwise-identical to the full [16384,16384] matmul).
    RECHECK_DELTA = np.float32(5e-4)
    W = 18
    chunk_base = (np.arange(NCAND, dtype=np.uint32) // 8 * CHUNK)[None, :]
    idx_rows = np.empty((N_SRC, K), dtype=np.int32)
    flagged = []
    for ci_ in range(NCORES):
        r = res.results[ci_]
        v = r["cval"].reshape(ROWS, NCAND)                          # rows t*128+p
        gi = (r["cidx"].reshape(ROWS, NCAND) + chunk_base).astype(np.int64)
        order = np.argsort(-v, axis=1, kind="stable")[:, :W]
        topv = np.take_along_axis(v, order, axis=1)                 # desc s == asc d2
        d2w = -2.0 * topv.astype(np.float64)
        near = (np.diff(d2w, axis=1) < RECHECK_DELTA).any(axis=1)
        # chunk-truncation risk: >=8 of top-16 slots from one chunk
        ch16 = (order[:, :K] >> 3)
        ch_sorted = np.sort(ch16, axis=1)
        same_run = np.zeros(ROWS, dtype=bool)
        run = np.ones(ROWS, dtype=np.int32)
        for j in range(1, K):
            run = np.where(ch_sorted[:, j] == ch_sorted[:, j - 1], run + 1, 1)
            same_run |= run >= 8
        rows_flagged = np.flatnonzero(near | same_run)
        flagged.extend((ci_ * ROWS + rows_flagged).tolist())
        idx_rows[ci_ * ROWS:(ci_ + 1) * ROWS] = np.take_along_axis(
            gi, order[:, :K], axis=1).astype(np.int32)

    global last_flag_count
    last_flag_count = len(flagged)
    if flagged:
        flagged = np.asarray(sorted(set(flagged)), dtype=np.int64)
        rows = flagged
        if rows.size < 64:  # M>=64 required for the bitwise-equal XLA kernel path
            pad = np.setdiff1d(np.arange(256, dtype=np.int64), rows)[: 64 - rows.size]
            rows = np.sort(np.concatenate([rows, pad]))
        with jax.default_device(cpu):
            dotx = np.asarray(jnp.asarray(np.ascontiguousarray(src[rows])) @ jnp.asarray(dst).T)
        d2x = (a[rows, None].astype(np.float32) - np.float32(2.0) * dotx) + c[None, :].astype(np.float32)
        keep = np.isin(rows, flagged)
        for rloc in np.flatnonzero(keep):
            row = rows[rloc]
            d2r = d2x[rloc]
            p = np.argpartition(d2r, 32)[:32]
            o = np.lexsort((p, d2r[p]))[:K]
            idx_rows[row] = p[o].astype(np.int32)

    # --- host: replay the O(E) tail with XLA-CPU ops (bitwise-matches reference) ---
    with jax.default_device(cpu):
        jsrc = jnp.asarray(src)
        jdst = jnp.asarray(dst)
        src_idx = jnp.broadcast_to(jnp.arange(N_SRC, dtype=jnp.int32)[:, None], (N_SRC, K))
        jidx = jnp.asarray(idx_rows)
        graph = jnp.stack([src_idx.reshape(-1), jidx.reshape(-1)], axis=0)
        likelihood = jnp.einsum("ed,ed->e", jsrc[graph[0]], jdst[graph[1]])
        mean = jnp.mean(likelihood)
        var = jnp.mean(jnp.square(likelihood - mean))
        logits = (likelihood - mean) * jax.lax.rsqrt(var + BN_EPS) * gamma[0] + beta[0]
        edge_weights = jnp.exp(logits)
        deg = jax.ops.segment_sum(edge_weights, graph[0], num_segments=N_SRC)
        edge_weights = edge_weights / (1e-12 + deg[graph[0]])
        graph_np = np.asarray(graph)
        ew_np = np.asarray(edge_weights[:, None])

    return graph_np, ew_np


# revision 12
# speedup vs baseline: 1.0164x; 1.0164x over previous
"""Trainium2 Bass kernel for dynamic kNN graph construction.

Reference computation (see problem):
  d2[i,j] = |src_i|^2 - 2 src_i.dst_j + |dst_j|^2   (fp32, [16384,16384] via GEMM)
  idx     = top-16 nearest dst per src row (ascending d2, stable ties)
  likelihood[e] = src_i . dst_j per edge, BatchNorm1d over all edges,
  exp -> edge weights, normalized by per-src-row degree sum.

Device strategy (8 NeuronCores, data-parallel over src rows, 2048 rows/core):
  - PE: bf16 hi/lo split GEMM (3 accumulating matmuls) + a K=6 matmul that
    folds -a/2 (per-row) and -c/2 (per-col, 3-term bf16 splits) into PSUM,
    so PSUM holds s~ = dot~ - a/2 - c/2 (~= -d2/2 up to ~1e-4)
  - ACT: plain PSUM -> SBUF copy
  - DVE: per-2048-chunk max8 + max_index -> 64 exact-f32 candidates
    (value + in-chunk idx) per row
  - host: merge 64 candidates -> top-16 with jax.lax.top_k tie semantics.
    Rows where device numerics could diverge from the reference are
    recomputed exactly: (a) near-ties (adjacent top-18 d2 gap < 5e-4,
    ~10x the worst-case device-vs-XLA numeric difference), (b) rows where
    one chunk supplied >=8 of the top-16 (candidate-truncation risk, makes
    the 8-per-chunk selection sound at any chunk width). For those rows the
    full d2 row is recomputed with an XLA-CPU matmul over gathered rows
    (M>=64 row-gathered matmuls are bitwise-identical to the reference's
    full [16384,16384] matmul - verified empirically).
  - host: replay the O(E) tail (gather-dot einsum, batchnorm, exp, degree
    normalization) with the same XLA-CPU ops the reference uses -> output
    is bitwise identical to the reference.
"""

import os
import sys

for _p in ("/opt/trn_rl_repo", "/root/.axon_site/_ro/trn_rl_repo"):
    if os.path.isdir(_p) and _p not in sys.path:
        sys.path.append(_p)

import numpy as np

N_SRC = 16384
N_DST = 16384
D = 128
K = 16
NCORES = 8
ROWS = N_SRC // NCORES          # 2048 src rows per core
NT = ROWS // 128                # 16 row-tiles of 128 rows
NJ = N_DST // 512               # 32 j-blocks of 512 cols
CHUNK = 4096                    # top-8 selection chunk width
NCH = N_DST // CHUNK            # 8 chunks per row
NCAND = NCH * 8                 # 64 candidates per row

BN_EPS = 1e-5

_cached_nc = None
last_exec_time_ns = None
last_results = None
last_flag_count = None


def _build_nc():
    """bf16 hi/lo split GEMM version.

    s~[i,j] = dot~[i,j] - a_i/2 - c_j/2 accumulated in one PSUM group:
      MM1 srcHi.T @ dstHi, MM2 srcHi.T @ dstLo, MM3 srcLo.T @ dstHi   (K=128)
      MM4 aug_lhs.T @ aug_rhs (K=6): rows 0-2 carry -c/2 (3-term bf16 split,
          ones weights), rows 3-5 carry -a/2 (3-term split, ones rhs)
    |s~ - s_exact| <= ~6e-5 (missing lo*lo term + split residuals), well under
    the host recheck margin RECHECK_DELTA=5e-4.
    ACT drains PSUM -> SBUF (plain copy); DVE does per-512-chunk max8 +
    max_index only.
    """
    global _cached_nc
    if _cached_nc is not None:
        return _cached_nc

    import concourse.bacc as bacc
    import concourse.tile as tile
    from concourse import mybir

    nc = bacc.Bacc("TRN2", target_bir_lowering=False, debug=False)
    bf16 = mybir.dt.bfloat16
    srcT_hi = nc.dram_tensor("srcT_hi", [D, ROWS], bf16, kind="ExternalInput").ap()
    srcT_lo = nc.dram_tensor("srcT_lo", [D, ROWS], bf16, kind="ExternalInput").ap()
    dstT_hi = nc.dram_tensor("dstT_hi", [D, N_DST], bf16, kind="ExternalInput").ap()
    dstT_lo = nc.dram_tensor("dstT_lo", [D, N_DST], bf16, kind="ExternalInput").ap()
    aug_lhs = nc.dram_tensor("aug_lhs", [6, ROWS], bf16, kind="ExternalInput").ap()
    aug_rhs = nc.dram_tensor("aug_rhs", [6, N_DST], bf16, kind="ExternalInput").ap()
    cval = nc.dram_tensor("cval", [NT, 128, NCAND], mybir.dt.float32, kind="ExternalOutput").ap()
    cidx = nc.dram_tensor("cidx", [NT, 128, NCAND], mybir.dt.uint32, kind="ExternalOutput").ap()

    with tile.TileContext(nc) as tc:
        with (
            tc.tile_pool(name="const", bufs=1) as cp,
            tc.tile_pool(name="psum", bufs=8, space="PSUM") as psp,
            tc.tile_pool(name="s", bufs=3) as sp,
            tc.tile_pool(name="cand", bufs=2) as cdp,
        ):
            sh_t = cp.tile([D, ROWS], bf16)
            sl_t = cp.tile([D, ROWS], bf16)
            dh_t = cp.tile([D, N_DST], bf16)
            dl_t = cp.tile([D, N_DST], bf16)
            al_t = cp.tile([128, ROWS], bf16)      # rows 0-5 used
            ar_t = cp.tile([128, N_DST], bf16)     # rows 0-5 used

            nc.sync.dma_start(sh_t[:], srcT_hi[:])
            nc.sync.dma_start(sl_t[:], srcT_lo[:])
            nc.sync.dma_start(al_t[0:6, :], aug_lhs[:])
            nc.sync.dma_start(ar_t[0:6, :], aug_rhs[:])
            NSPLIT = 8
            w = N_DST // NSPLIT
            for i in range(NSPLIT):
                sl = slice(i * w, (i + 1) * w)
                nc.sync.dma_start(dh_t[:, sl], dstT_hi[:, sl])
                nc.sync.dma_start(dl_t[:, sl], dstT_lo[:, sl])

            for t in range(NT):
                cv = cdp.tile([128, NCAND], mybir.dt.float32, name="cv", tag="cv")
                ci = cdp.tile([128, NCAND], mybir.dt.uint32, name="ci", tag="ci")
                tsl = slice(t * 128, (t + 1) * 128)
                # process j-blocks in groups of 8 (one PSUM bank each) so each
                # stationary operand is loaded once per group of 8 matmuls
                for g in range(NJ // 8):
                    pss = []
                    jbs = list(range(g * 8, g * 8 + 8))
                    for jb in jbs:
                        pss.append(psp.tile([128, 512], mybir.dt.float32, name="ps", tag="ps"))
                    for jb, ps in zip(jbs, pss):
                        jsl = slice(jb * 512, (jb + 1) * 512)
                        nc.tensor.matmul(ps[:], sh_t[:, tsl], dh_t[:, jsl],
                                         start=True, stop=False)
                    for jb, ps in zip(jbs, pss):
                        jsl = slice(jb * 512, (jb + 1) * 512)
                        nc.tensor.matmul(ps[:], sh_t[:, tsl], dl_t[:, jsl],
                                         start=False, stop=False, skip_group_check=True)
                    for jb, ps in zip(jbs, pss):
                        jsl = slice(jb * 512, (jb + 1) * 512)
                        nc.tensor.matmul(ps[:], sl_t[:, tsl], dh_t[:, jsl],
                                         start=False, stop=False, skip_group_check=True)
                    for jb, ps in zip(jbs, pss):
                        jsl = slice(jb * 512, (jb + 1) * 512)
                        nc.tensor.matmul(ps[:], al_t[0:6, tsl], ar_t[0:6, jsl],
                                         start=False, stop=True, skip_group_check=True)
                    # 8 j-blocks -> one 4096-wide selection chunk
                    sj = sp.tile([128, CHUNK], mybir.dt.float32, name="sj", tag="sj")
                    for bi, (jb, ps) in enumerate(zip(jbs, pss)):
                        nc.scalar.activation(sj[:, bi * 512:bi * 512 + 512],
                                             ps[:], mybir.ActivationFunctionType.Copy)
                    csl = slice(g * 8, g * 8 + 8)
                    nc.vector.max(cv[:, csl], sj[:])
                    nc.vector.max_index(ci[:, csl], cv[:, csl], sj[:])
                nc.sync.dma_start(cval[t], cv[:])
                nc.sync.dma_start(cidx[t], ci[:])

    nc.compile()
    _cached_nc = nc
    return nc


def _split_bf16(x, terms):
    import ml_dtypes
    parts = []
    r = x.astype(np.float32)
    for _ in range(terms):
        h = r.astype(ml_dtypes.bfloat16)
        parts.append(h)
        r = r - h.astype(np.float32)
    return parts


def _cpu_device():
    import jax
    return jax.devices("cpu")[0]


def kernel(src_embeddings, dst_embeddings, gamma, beta, k):
    global last_exec_time_ns, last_results

    import jax
    import jax.numpy as jnp
    from concourse.bass_utils import run_bass_kernel_spmd

    src = np.asarray(src_embeddings, dtype=np.float32)
    dst = np.asarray(dst_embeddings, dtype=np.float32)
    gamma = np.asarray(gamma, dtype=np.float32)
    beta = np.asarray(beta, dtype=np.float32)
    kk = int(np.asarray(k))
    assert src.shape == (N_SRC, D) and dst.shape == (N_DST, D) and kk == K

    cpu = _cpu_device()
    with jax.default_device(cpu):
        a = np.asarray(jnp.sum(jnp.asarray(src) * jnp.asarray(src), axis=-1))
        c = np.asarray(jnp.sum(jnp.asarray(dst) * jnp.asarray(dst), axis=-1))

    dstT = np.ascontiguousarray(dst.T)
    d_hi, d_lo = _split_bf16(dstT, 2)
    nc1, nc2, nc3 = _split_bf16(-0.5 * c, 3)
    import ml_dtypes
    ones_n = np.ones((1, N_DST), ml_dtypes.bfloat16)
    aug_rhs = np.ascontiguousarray(
        np.concatenate([nc1[None, :], nc2[None, :], nc3[None, :],
                        ones_n, ones_n, ones_n], axis=0))

    in_maps = []
    for ci_ in range(NCORES):
        rows = slice(ci_ * ROWS, (ci_ + 1) * ROWS)
        srcT_c = np.ascontiguousarray(src[rows].T)
        s_hi, s_lo = _split_bf16(srcT_c, 2)
        na1, na2, na3 = _split_bf16(-0.5 * a[rows], 3)
        ones_r = np.ones((1, ROWS), ml_dtypes.bfloat16)
        aug_lhs = np.ascontiguousarray(
            np.concatenate([ones_r, ones_r, ones_r,
                            na1[None, :], na2[None, :], na3[None, :]], axis=0))
        in_maps.append({
            "srcT_hi": s_hi, "srcT_lo": s_lo,
            "dstT_hi": d_hi, "dstT_lo": d_lo,
            "aug_lhs": aug_lhs, "aug_rhs": aug_rhs,
        })

    nc = _build_nc()
    trace = bool(os.environ.get("KNN_BASS_TRACE"))
    res = run_bass_kernel_spmd(nc, in_maps, core_ids=list(range(NCORES)), trace=trace)
    last_exec_time_ns = res.exec_time_ns
    last_results = res

    # --- host: merge NCAND candidates -> top-16 per row ---
    # Device values are s~ ~= -d2/2 (device-vs-reference difference <~1e-4 in
    # d2 units). Flag rows whose top-18 has any adjacent d2 gap <
    # RECHECK_DELTA (plus rows where one chunk contributed >=8 of the top 16
    # - candidate truncation risk) and recompute those rows' d2
    # bitwise-as-reference on XLA-CPU (row-gathered matmuls with M>=64 are
    # bitwise-identical to the full [16384,16384] matmul).
    RECHECK_DELTA = np.float32(5e-4)
    W = 18
    chunk_base = (np.arange(NCAND, dtype=np.uint32) // 8 * CHUNK)[None, :]
    idx_rows = np.empty((N_SRC, K), dtype=np.int32)
    flagged = []
    for ci_ in range(NCORES):
        r = res.results[ci_]
        v = r["cval"].reshape(ROWS, NCAND)                          # rows t*128+p
        gi = (r["cidx"].reshape(ROWS, NCAND) + chunk_base).astype(np.int64)
        order = np.argsort(-v, axis=1, kind="stable")[:, :W]
        topv = np.take_along_axis(v, order, axis=1)                 # desc s == asc d2
        d2w = -2.0 * topv.astype(np.float64)
        near = (np.diff(d2w, axis=1) < RECHECK_DELTA).any(axis=1)
        # chunk-truncation risk: >=8 of top-16 slots from one chunk
        ch16 = (order[:, :K] >> 3)
        ch_sorted = np.sort(ch16, axis=1)
        same_run = np.zeros(ROWS, dtype=bool)
        run = np.ones(ROWS, dtype=np.int32)
        for j in range(1, K):
            run = np.where(ch_sorted[:, j] == ch_sorted[:, j - 1], run + 1, 1)
            same_run |= run >= 8
        rows_flagged = np.flatnonzero(near | same_run)
        flagged.extend((ci_ * ROWS + rows_flagged).tolist())
        idx_rows[ci_ * ROWS:(ci_ + 1) * ROWS] = np.take_along_axis(
            gi, order[:, :K], axis=1).astype(np.int32)

    global last_flag_count
    last_flag_count = len(flagged)
    if flagged:
        flagged = np.asarray(sorted(set(flagged)), dtype=np.int64)
        rows = flagged
        if rows.size < 64:  # M>=64 required for the bitwise-equal XLA kernel path
            pad = np.setdiff1d(np.arange(256, dtype=np.int64), rows)[: 64 - rows.size]
            rows = np.sort(np.concatenate([rows, pad]))
        with jax.default_device(cpu):
            dotx = np.asarray(jnp.asarray(np.ascontiguousarray(src[rows])) @ jnp.asarray(dst).T)
        d2x = (a[rows, None].astype(np.float32) - np.float32(2.0) * dotx) + c[None, :].astype(np.float32)
        keep = np.isin(rows, flagged)
        for rloc in np.flatnonzero(keep):
            row = rows[rloc]
            d2r = d2x[rloc]
            p = np.argpartition(d2r, 32)[:32]
            o = np.lexsort((p, d2r[p]))[:K]
            idx_rows[row] = p[o].astype(np.int32)

    # --- host: replay the O(E) tail with XLA-CPU ops (bitwise-matches reference) ---
    with jax.default_device(cpu):
        jsrc = jnp.asarray(src)
        jdst = jnp.asarray(dst)
        src_idx = jnp.broadcast_to(jnp.arange(N_SRC, dtype=jnp.int32)[:, None], (N_SRC, K))
        jidx = jnp.asarray(idx_rows)
        graph = jnp.stack([src_idx.reshape(-1), jidx.reshape(-1)], axis=0)
        likelihood = jnp.einsum("ed,ed->e", jsrc[graph[0]], jdst[graph[1]])
        mean = jnp.mean(likelihood)
        var = jnp.mean(jnp.square(likelihood - mean))
        logits = (likelihood - mean) * jax.lax.rsqrt(var + BN_EPS) * gamma[0] + beta[0]
        edge_weights = jnp.exp(logits)
        deg = jax.ops.segment_sum(edge_weights, graph[0], num_segments=N_SRC)
        edge_weights = edge_weights / (1e-12 + deg[graph[0]])
        graph_np = np.asarray(graph)
        ew_np = np.asarray(edge_weights[:, None])

    return graph_np, ew_np
